# revision 44
# baseline (speedup 1.0000x reference)
"""HSTU attention Trainium2 kernel (fp8 DoubleRow).

Sharding: 8 cores = 4 batches x 2 head-groups; core d = (batch d//2, group
d%2) computes its 4 heads end-to-end and a partial output projection; the
host sums the two group partials per batch and adds the residual.

Numerics (validated host-side and on HW: end-to-end rel err ~1.0e-3 vs the
2e-2 gate):
  * The LayerNorm affine (g, b) is applied on the host; the device gets
    x_norm^T in fp8e4m3. All four matmul stages run as fp8 DoubleRow
    (2 k-subtiles per instruction, 0.5 PE cycles/row):
      - QKV:    w8 (x32) [128,2,128] x xt8 [128,2,512] chunks, K=1024
      - scores: k-subtile pair = [real k-block | zeros] (zero-padded DR -
        half the cycles of bf16 even with the wasted half)
      - attn*v: k-subtile pair = two adjacent 128-token j-blocks; the
        stationary operand carries BOTH heads of a pair (128 out rows) since
        DR rejects output base-partition 64
      - out-proj: k-subtile pair = the two head-pair chunks (K=256)
  * scores PSUM = (32q).(32k) = 8192*s; silu applied with scale 2^-13.
    Causal masking happens IN PSUM by accumulating -B (B=2^17) onto
    below-diagonal regions via small mask matmuls (strict-upper -B x identity
    for the diagonal band, a rank-1 fill for fully-masked blocks):
    silu(s-16) rounds to 0 in fp8e4m3, so attn*v sees exact zeros and no
    vector-engine masking is needed.
  * exp(p) ~ 1+p (p = silu/n ~ 1e-3, as in the reference's masked-softmax
    linearization): numerator = prefix-sum(v) (DVE scan) + (1/n)*(silu x v)
    matmuls. Denominator: |sum_j silu/n| <= 2.8e-4 of (i+1), far below fp8
    noise, so d ~ (i+1): out = (spref + ap/n) * 16/(i+1) with 16/(i+1) a
    static bf16 broadcast tile. outT is 16*out in fp8; wo is x64 fp8; the
    host unscales the bf16 partial by 2^-10. b_qkv is zero in this problem.

Engine busy per core (cost model): Act (silu, the bottleneck) ~80us, DVE
~61us, PE ~44us, Pool ~25us, DMA ~22us; total 98us vs the baseline's 256us.
Lead-in DMAs split across both HWDGE queues (SP + the idle Act engine).
Emission interleaves P1 (QKV chunk c4), P2 (attention stage ic=c4: pairs
p<=2*ic+1 of 128-row j-block pairs against the 512-col output chunk ic) and
P3 (out-proj of finished chunks) so Act stays fed end to end; attn*v matmuls
trail their silu by one unit so the in-order PE queue never blocks on the
silu it just requested. PSUM: scores ring 3x2 banks, attn accumulators 1x1,
scratch ring 1x1 = 8 banks exactly.
"""

import numpy as np
from contextlib import ExitStack

B, N_FULL, D = 4, 2048, 1024
H, ATT, LIN = 8, 64, 64
EPS = 1e-5
NCORES = 8
NEGB = 131072.0  # -B for PSUM causal masking; silu((psum-B)*2^-13) == 0 in fp8


def build_nc(n=N_FULL, reps=1, dbg=False):
    """Single-core SPMD Bass program; all 8 cores run it on different slices."""
    import contextlib
    import concourse.bacc as bacc
    import concourse.tile as tile
    from concourse import mybir

    f8 = mybir.dt.float8e4
    bf = mybir.dt.bfloat16
    f32 = mybir.dt.float32
    AF = mybir.ActivationFunctionType
    ALU = mybir.AluOpType
    DR = mybir.MatmulPerfMode.DoubleRow

    nstg = n // 512  # 512-col stages (= c4 chunks)

    nc = bacc.Bacc("TRN2", target_bir_lowering=False, debug=False)

    xt8 = nc.dram_tensor("xt8", [D, n], f8, kind="ExternalInput").ap()
    w8 = nc.dram_tensor("w8", [D, 768], f8, kind="ExternalInput").ap()
    wo8 = nc.dram_tensor("wo8", [256, D], f8, kind="ExternalInput").ap()
    cbf = nc.dram_tensor("cbf", [128, n + 512], bf, kind="ExternalInput").ap()
    yout = nc.dram_tensor("yout", [n, D], bf, kind="ExternalOutput").ap()
    if dbg:
        dq8 = nc.dram_tensor("dq8", [128, 2, 2, n], f8, kind="ExternalOutput").ap()
        dk8 = nc.dram_tensor("dk8", [128, 2, 2, n], f8, kind="ExternalOutput").ap()
        dvb = nc.dram_tensor("dvb", [128, 2, n], bf, kind="ExternalOutput").ap()
        dsp = nc.dram_tensor("dsp", [128, 2, n], bf, kind="ExternalOutput").ap()
        dva = nc.dram_tensor("dva", [128, 2, 8, 2, 128], f8, kind="ExternalOutput").ap()
        dot = nc.dram_tensor("dot", [128, 2, n], f8, kind="ExternalOutput").ap()
        dtt = nc.dram_tensor("dtt", [128, 2, 512], f8, kind="ExternalOutput").ap()

    dbgt = {}
    if dbg:
        dbgt = {"dq8": dq8, "dk8": dk8, "dvb": dvb, "dsp": dsp, "dva": dva,
                "dot": dot, "dtt": dtt}
    with tile.TileContext(nc) as tc, ExitStack() as ctx:
        wpool = ctx.enter_context(tc.tile_pool(name="wpool", bufs=1))
        big = ctx.enter_context(tc.tile_pool(name="big", bufs=1))
        xtpool = ctx.enter_context(tc.tile_pool(name="xtpool", bufs=3))
        ttpool = ctx.enter_context(tc.tile_pool(name="ttpool", bufs=12))
        oupool = ctx.enter_context(tc.tile_pool(name="oupool", bufs=4))
        yspool = ctx.enter_context(tc.tile_pool(name="yspool", bufs=3))
        psp = ctx.enter_context(tc.tile_pool(name="psp", bufs=1, space="PSUM"))

        # ---- weights / constants (DMA once) ----
        w_sb = wpool.tile([128, 8, 768], f8)
        w8r = w8.rearrange("(kc p) c -> p kc c", p=128)
        nc.sync.dma_start(out=w_sb[:, :, 0:512], in_=w8r[:, :, 0:512])
        wo_sb = wpool.tile([128, 2, D], f8)
        cbf_sb = wpool.tile([128, n + 512], bf)
        ubc_sb = cbf_sb[:, 0:n]
        mtri_sb = cbf_sb[:, n:n + 128]
        identb_sb = cbf_sb[:, n + 128:n + 256]
        aux_sb = cbf_sb[0:1, n + 256:n + 512]
        def late_const_dmas():
            nc.sync.dma_start(out=cbf_sb, in_=cbf)
            nc.sync.dma_start(out=w_sb[:, :, 512:768], in_=w8r[:, :, 512:768])
            nc.sync.dma_start(out=wo_sb, in_=wo8.rearrange("(c p) d -> p c d", p=128))

        # ---- persistent activations ----
        # q8/k8: [part, m-chunk(head pair), DR-subtile slot, col]; slot 1 is
        # zeroed once so the scores DoubleRow contracts [real | zeros].
        q8 = big.tile([128, 2, 2, n], f8)
        k8 = big.tile([128, 2, 2, n], f8)
        vb16 = big.tile([128, 2, n], bf)       # v (unscaled), transposed layout
        spref = big.tile([128, 2, n], bf)      # prefix sums of v
        vaug = big.tile([128, 2, 8, 2, 128], f8)  # v natural, per (pair-of-heads c, jb)
        outT = big.tile([128, 2, n], f8)       # 16 * attention output, transposed
        nc.gpsimd.memset(q8[:, :, 1, :], 0.0)
        nc.gpsimd.memset(k8[:, :, 1, :], 0.0)

        rep_ctx = tc.For_i(0, reps, 1) if reps > 1 else contextlib.nullcontext()
        ctx.enter_context(rep_ctx)

        def p1_items(c4, ptag="one", pbufs=1):
            """QKV^T chunk c4 as a list of closures (PE filler work)."""
            cols = slice(c4 * 512, (c4 + 1) * 512)
            xtc = xtpool.tile([128, 8, 512], f8, tag="xt", name=f"xtc_{c4}")

            def dma_item():
                eng = nc.scalar if c4 == 0 else nc.sync
                eng.dma_start(
                    out=xtc,
                    in_=xt8.rearrange("(kc p) c -> p kc c", p=128)[:, :, cols])

            def qkv_item(m):
                def run():
                    qps = psp.tile([128, 512], f32, tag=ptag, bufs=pbufs,
                                   name=f"qkv_{c4}_{m}")
                    for kk in range(4):
                        nc.tensor.matmul(out=qps,
                                         lhsT=w_sb[:, 2 * kk:2 * kk + 2, m * 128:(m + 1) * 128],
                                         rhs=xtc[:, 2 * kk:2 * kk + 2, :],
                                         start=(kk == 0), stop=(kk == 3), perf_mode=DR)
                    if m < 2:
                        nc.vector.tensor_copy(out=q8[:, m, 0, cols], in_=qps)
                    elif m < 4:
                        nc.vector.tensor_copy(out=k8[:, m - 2, 0, cols], in_=qps)
                    else:
                        nc.vector.tensor_scalar_mul(out=vb16[:, m - 4, cols], in0=qps,
                                                    scalar1=2.0 ** -5)
                return run

            def sct_item(h):
                def run():
                    ro, c = 64 * (h % 2), h // 2
                    init = 0.0 if c4 == 0 else spref[ro:ro + 64, c, c4 * 512 - 1:c4 * 512]
                    nc.vector.tensor_tensor_scan(out=spref[ro:ro + 64, c, cols],
                                                 data0=vb16[ro:ro + 64, c, cols],
                                                 data1=vb16[ro:ro + 64, c, cols],
                                                 initial=init, op0=ALU.add,
                                                 op1=ALU.bypass)
                    tp = psp.tile([128, 2, 2, 64], bf, tag=ptag, bufs=pbufs,
                                  name=f"tp_{c4}_{h}")
                    for bi in range(4):
                        jb = 4 * c4 + bi
                        nc.tensor.transpose(out=tp[:, bi // 2, bi % 2, :],
                                            in_=vb16[ro:ro + 64, c, jb * 128:(jb + 1) * 128],
                                            identity=identb_sb[ro:ro + 64, ro:ro + 64])
                    nc.vector.tensor_copy(
                        out=vaug[:, c, 2 * c4:2 * c4 + 2, :, ro:ro + 64], in_=tp)
                return run

            # q/k of head-pair 0 first: stage (h0, p0) depends only on m0+m2
            return [dma_item] + [qkv_item(m) for m in (0, 2, 1, 3, 4, 5)] + \
                [sct_item(h) for h in range(4)]

        def p3_items(a, ptag="one", pbufs=1, tail=False):
            """Output projection for token chunk a: 8 half-block closures."""
            items = []
            span = 2
            for g in range(4 // span):
                ib0 = 4 * a + span * g
                ysb = yspool.tile([128, span, 1024], bf, tag="ys",
                                  name=f"ysb_{ib0}")

                def half_item(ib, n2, ysb, ib0=ib0):
                    def run():
                        ypt = psp.tile([128, 512], f32, tag=ptag, bufs=pbufs,
                                       name=f"ypt_{ib}_{n2}")
                        nc.tensor.matmul(out=ypt,
                                         lhsT=outT[:, :, ib * 128:(ib + 1) * 128],
                                         rhs=wo_sb[:, :, n2 * 512:(n2 + 1) * 512],
                                         start=True, stop=True, perf_mode=DR)
                        half = ysb[:, ib - ib0, n2 * 512:(n2 + 1) * 512]
                        if tail and (ib + n2) % 2 == 0:
                            nc.scalar.copy(out=half, in_=ypt)
                        else:
                            nc.vector.tensor_copy(out=half, in_=ypt)
                        if ib == ib0 + span - 1 and n2 == 1:
                            nc.sync.dma_start(
                                out=yout[ib0 * 128:(ib0 + span) * 128, :].rearrange(
                                    "(i p) d -> p i d", p=128),
                                in_=ysb)
                    return run

                for ib in range(ib0, ib0 + span):
                    items += [half_item(ib, 0, ysb), half_item(ib, 1, ysb)]
            return items

        def stage_units(ic, last=False):
            """Attention units for output chunk ic. Each unit = (front, back):
            front = scores+masks+silu, back = attn*v matmul (+finalize on the
            head's last pair). The weaver emits back one unit late so the PE
            stream never blocks on the silu it just requested."""
            base = ic * 512
            pmax = min(7, 2 * ic + 1)
            units = []
            ap_ref = {}
            for h in range(4):
                ro, c = 64 * (h % 2), h // 2
                for p in range(pmax + 1):
                    diag = (p // 2 == ic)
                    lo = 256 * (p % 2) if diag else 0
                    tt = ttpool.tile([128, 2, 512], f8, tag="tt",
                                     name=f"tt_{h}_{p}_{ic}")

                    def front(h=h, p=p, ro=ro, c=c, lo=lo, diag=diag, tt=tt):
                        sps = psp.tile([128, 2, 512], f32, tag="big2", bufs=3,
                                       name=f"sps_{h}_{p}_{ic}")
                        nc.tensor.matmul(out=sps[:, 0, lo:512],
                                         lhsT=k8[ro:ro + 64, c, :, 2 * p * 128:(2 * p + 1) * 128],
                                         rhs=q8[ro:ro + 64, c, :, base + lo:base + 512],
                                         start=True, stop=not diag, perf_mode=DR,
                                         skip_group_check=True)
                        if diag:
                            nc.tensor.matmul(out=sps[:, 0, lo:lo + 128], lhsT=mtri_sb,
                                             rhs=identb_sb, start=False, stop=True,
                                             skip_group_check=True)
                            nc.tensor.matmul(out=sps[:, 1, lo:lo + 128],
                                             lhsT=aux_sb[0:1, 0:128],
                                             rhs=aux_sb[0:1, 128:256],
                                             start=True, stop=True,
                                             skip_group_check=True)
                            nc.tensor.matmul(out=sps[:, 1, lo + 128:512],
                                             lhsT=k8[ro:ro + 64, c, :, (2 * p + 1) * 128:(2 * p + 2) * 128],
                                             rhs=q8[ro:ro + 64, c, :, base + lo + 128:base + 512],
                                             start=True, stop=False, perf_mode=DR,
                                             skip_group_check=True)
                            nc.tensor.matmul(out=sps[:, 1, lo + 128:lo + 256],
                                             lhsT=mtri_sb, rhs=identb_sb,
                                             start=False, stop=True,
                                             skip_group_check=True)
                        else:
                            nc.tensor.matmul(out=sps[:, 1, 0:512],
                                             lhsT=k8[ro:ro + 64, c, :, (2 * p + 1) * 128:(2 * p + 2) * 128],
                                             rhs=q8[ro:ro + 64, c, :, base:base + 512],
                                             start=True, stop=True, perf_mode=DR,
                                             skip_group_check=True)
                        nc.scalar.activation(out=tt[:, :, lo:512],
                                             in_=sps[:, :, lo:512],
                                             func=AF.Silu, scale=2.0 ** -13)
                        if dbg and h == 0 and p == 0 and ic == 0:
                            nc.sync.dma_start(out=dbgt["dtt"], in_=tt)

                    def back(h=h, p=p, ro=ro, c=c, lo=lo, tt=tt):
                        if p == 0:
                            ap_ref[h] = psp.tile([128, 512], f32, tag="ap", bufs=1,
                                                 name=f"ap_{h}_{ic}")
                        nc.tensor.matmul(out=ap_ref[h][:, lo:512],
                                         lhsT=vaug[:, c, p, :, :],
                                         rhs=tt[:, :, lo:512],
                                         start=(p == 0), stop=(p == pmax),
                                         perf_mode=DR, skip_group_check=True)
                        if p == pmax:
                            # out = (prefix(v) + ap/n) * 16/(i+1), fp8 (x16).
                            # The very last finalize is split in halves on DVE
                            # so the trailing out-proj starts half a tile early.
                            crit = last and h == 3
                            ou = oupool.tile([128, 512], bf, tag="ou",
                                             name=f"ou_{h}_{ic}")
                            mul_eng = nc.vector if crit else nc.gpsimd
                            for s0, s1 in ([(0, 256), (256, 512)] if crit
                                           else [(0, 512)]):
                                nc.vector.scalar_tensor_tensor(
                                    out=ou[ro:ro + 64, s0:s1],
                                    in0=ap_ref[h][ro:ro + 64, s0:s1],
                                    scalar=1.0 / n,
                                    in1=spref[ro:ro + 64, c, base + s0:base + s1],
                                    op0=ALU.mult, op1=ALU.add)
                                mul_eng.tensor_mul(
                                    out=outT[ro:ro + 64, c, base + s0:base + s1],
                                    in0=ou[ro:ro + 64, s0:s1],
                                    in1=ubc_sb[ro:ro + 64, base + s0:base + s1])

                    units.append((front, back))
            return units

        pending_back = [None]

        def weave(units, fillers):
            """Emit units with backs delayed one unit; spread fillers evenly."""
            nf, nu = len(fillers), max(1, len(units))
            fi = 0
            for i, (front, back) in enumerate(units):
                front()
                if pending_back[0] is not None:
                    pending_back[0]()
                pending_back[0] = back
                want = (i + 1) * nf // nu
                while fi < want:
                    fillers[fi]()
                    fi += 1
            while fi < nf:
                fillers[fi]()
                fi += 1

        # Stage 0 is folded into P1(0): fronts only need the m0/m2 (and
        # m1/m3) drains, so they start as soon as those chains land; backs
        # (which need vaug/scan) are deferred past the sct items, interleaved
        # with P1(1). This pulls the first silu ~5us earlier.
        p10 = p1_items(0, ptag="big2", pbufs=3)
        for item in p10[:3]:   # xtc DMA, qkv m0, qkv m2
            item()
        late_const_dmas()
        su0 = stage_units(0)
        rest = p10[3:]
        for i, (front, _) in enumerate(su0):
            front()
            if i < len(rest):
                rest[i]()
        for item in rest[len(su0):]:
            item()
        p11 = p1_items(1)
        for i, (_, back) in enumerate(su0):
            back()
            j0, j1 = i * len(p11) // len(su0), (i + 1) * len(p11) // len(su0)
            for item in p11[j0:j1]:
                item()
        for ic in range(1, nstg):
            fillers = []
            if ic + 1 < nstg:
                fillers += p1_items(ic + 1)
            fillers += p3_items(ic - 1)
            weave(stage_units(ic, last=(ic == nstg - 1)), fillers)
        if pending_back[0] is not None:
            pending_back[0]()
        for item in p3_items(nstg - 1, ptag="big2", pbufs=3, tail=True):
            item()
        if dbg:
            nc.sync.dma_start(out=dbgt["dq8"], in_=q8)
            nc.sync.dma_start(out=dbgt["dk8"], in_=k8)
            nc.sync.dma_start(out=dbgt["dvb"], in_=vb16)
            nc.sync.dma_start(out=dbgt["dsp"], in_=spref)
            nc.sync.dma_start(out=dbgt["dva"], in_=vaug)
            nc.sync.dma_start(out=dbgt["dot"], in_=outT)

    nc.compile()
    return nc


def prep_in_maps(x, ln_g, ln_b, w_qkv, w_out, n=N_FULL, n_batches=B):
    """Host-side prep: LayerNorm, weight fold/reorder, fp8 casts, per-core dicts."""
    import ml_dtypes
    f8 = ml_dtypes.float8_e4m3fn
    bf16 = ml_dtypes.bfloat16

    x = np.asarray(x, np.float32)
    mu = x.mean(-1, keepdims=True)
    var = ((x - mu) ** 2).mean(-1, keepdims=True)
    xn = (x - mu) / np.sqrt(var + EPS) * np.asarray(ln_g, np.float32) \
        + np.asarray(ln_b, np.float32)
    w_qkv = np.asarray(w_qkv, np.float32)
    w_out = np.asarray(w_out, np.float32)

    idx = np.arange(128)
    # packed bf16 constants [128, n+512]: ubc | mtri | identb | aux(-B, ones)
    cbf = np.zeros((128, n + 512), np.float32)
    cbf[:, 0:n] = 16.0 / np.arange(1, n + 1, dtype=np.float64)[None, :]
    cbf[:, n:n + 128] = np.where(idx[None, :] > idx[:, None], -NEGB, 0.0)
    cbf[:, n + 128:n + 256] = np.eye(128)
    cbf[0, n + 256:n + 384] = -NEGB
    cbf[0, n + 384:n + 512] = 1.0
    cbf = cbf.astype(bf16)

    in_maps = []
    for d in range(2 * n_batches):
        b, g = divmod(d, 2)
        # m-chunk neuron order: m0 q h01 | m1 q h23 | m2 k h01 | m3 k h23 | m4 v h01 | m5 v h23
        order = []
        for off in (0, 64, 128):  # q, k, v row offsets within a head's 256 rows
            for c in range(2):
                for i in (0, 1):
                    hh = g * 4 + 2 * c + i
                    order += list(range(hh * 256 + off, hh * 256 + off + 64))
        w8 = np.ascontiguousarray((w_qkv[order, :] * 32.0).T).astype(f8)  # [1024, 768]
        wo8 = np.ascontiguousarray(w_out[:, g * 256:(g + 1) * 256].T * 64.0).astype(f8)
        in_maps.append({
            "xt8": np.ascontiguousarray(xn[b].T).astype(f8),
            "w8": w8,
            "wo8": wo8,
            "cbf": cbf,
        })
    return in_maps


_cached_nc = None


def kernel(x, attention_mask, ln_g, ln_b, w_qkv, b_qkv, w_out, b_out):
    """Full-input entry point: shards across 8 NeuronCores, returns full output."""
    global _cached_nc
    from concourse.bass_utils import run_bass_kernel_spmd

    if _cached_nc is None:
        _cached_nc = build_nc(N_FULL)
    nc = _cached_nc

    in_maps = prep_in_maps(x, ln_g, ln_b, w_qkv, w_out)
    res = run_bass_kernel_spmd(nc, in_maps, core_ids=list(range(NCORES)))

    y = np.asarray(x, np.float32) + np.asarray(b_out, np.float32)[None, None, :]
    for d in range(NCORES):
        y[d // 2] += res.results[d]["yout"].astype(np.float32) * 2.0 ** -10
    return y


# revision 49
# speedup vs baseline: 1.0075x; 1.0075x over previous
"""HSTU attention Trainium2 kernel (fp8 DoubleRow).

Sharding: 8 cores = 4 batches x 2 head-groups; core d = (batch d//2, group
d%2) computes its 4 heads end-to-end and a partial output projection; the
host sums the two group partials per batch and adds the residual.

Numerics (validated host-side and on HW: end-to-end rel err ~1.0e-3 vs the
2e-2 gate):
  * The LayerNorm affine (g, b) is applied on the host; the device gets
    x_norm^T in fp8e4m3. All four matmul stages run as fp8 DoubleRow
    (2 k-subtiles per instruction, 0.5 PE cycles/row):
      - QKV:    w8 (x32) [128,2,128] x xt8 [128,2,512] chunks, K=1024
      - scores: k-subtile pair = [real k-block | zeros] (zero-padded DR -
        half the cycles of bf16 even with the wasted half)
      - attn*v: k-subtile pair = two adjacent 128-token j-blocks; the
        stationary operand carries BOTH heads of a pair (128 out rows) since
        DR rejects output base-partition 64
      - out-proj: k-subtile pair = the two head-pair chunks (K=256)
  * scores PSUM = (32q).(32k) = 8192*s; silu applied with scale 2^-13.
    Causal masking happens IN PSUM by accumulating -B (B=2^17) onto
    below-diagonal regions via small mask matmuls (strict-upper -B x identity
    for the diagonal band, a rank-1 fill for fully-masked blocks):
    silu(s-16) rounds to 0 in fp8e4m3, so attn*v sees exact zeros and no
    vector-engine masking is needed.
  * exp(p) ~ 1+p (p = silu/n ~ 1e-3, as in the reference's masked-softmax
    linearization): numerator = prefix-sum(v) (DVE scan) + (1/n)*(silu x v)
    matmuls. Denominator: |sum_j silu/n| <= 2.8e-4 of (i+1), far below fp8
    noise, so d ~ (i+1): out = (spref + ap/n) * 16/(i+1) with 16/(i+1) a
    static bf16 broadcast tile. outT is 16*out in fp8; wo is x64 fp8; the
    host unscales the bf16 partial by 2^-10. b_qkv is zero in this problem.

Engine busy per core (cost model): Act (silu, the bottleneck) ~80us, DVE
~61us, PE ~44us, Pool ~25us, DMA ~22us; total 98us vs the baseline's 256us.
Lead-in DMAs split across both HWDGE queues (SP + the idle Act engine).
Emission interleaves P1 (QKV chunk c4), P2 (attention stage ic=c4: pairs
p<=2*ic+1 of 128-row j-block pairs against the 512-col output chunk ic) and
P3 (out-proj of finished chunks) so Act stays fed end to end; attn*v matmuls
trail their silu by one unit so the in-order PE queue never blocks on the
silu it just requested. PSUM: scores ring 3x2 banks, attn accumulators 1x1,
scratch ring 1x1 = 8 banks exactly.
"""

import numpy as np
from contextlib import ExitStack

B, N_FULL, D = 4, 2048, 1024
H, ATT, LIN = 8, 64, 64
EPS = 1e-5
NCORES = 8
NEGB = 131072.0  # -B for PSUM causal masking; silu((psum-B)*2^-13) == 0 in fp8


def build_nc(n=N_FULL, reps=1, dbg=False):
    """Single-core SPMD Bass program; all 8 cores run it on different slices."""
    import contextlib
    import concourse.bacc as bacc
    import concourse.tile as tile
    from concourse import mybir

    f8 = mybir.dt.float8e4
    bf = mybir.dt.bfloat16
    f32 = mybir.dt.float32
    AF = mybir.ActivationFunctionType
    ALU = mybir.AluOpType
    DR = mybir.MatmulPerfMode.DoubleRow

    nstg = n // 512  # 512-col stages (= c4 chunks)

    nc = bacc.Bacc("TRN2", target_bir_lowering=False, debug=False)

    xt8 = nc.dram_tensor("xt8", [D, n], f8, kind="ExternalInput").ap()
    w8 = nc.dram_tensor("w8", [D, 768], f8, kind="ExternalInput").ap()
    wo8 = nc.dram_tensor("wo8", [256, D], f8, kind="ExternalInput").ap()
    cbf = nc.dram_tensor("cbf", [128, n + 512], bf, kind="ExternalInput").ap()
    yout = nc.dram_tensor("yout", [n, D], f8, kind="ExternalOutput").ap()
    if dbg:
        dq8 = nc.dram_tensor("dq8", [128, 2, 2, n], f8, kind="ExternalOutput").ap()
        dk8 = nc.dram_tensor("dk8", [128, 2, 2, n], f8, kind="ExternalOutput").ap()
        dvb = nc.dram_tensor("dvb", [128, 2, n], bf, kind="ExternalOutput").ap()
        dsp = nc.dram_tensor("dsp", [128, 2, n], bf, kind="ExternalOutput").ap()
        dva = nc.dram_tensor("dva", [128, 2, 8, 2, 128], f8, kind="ExternalOutput").ap()
        dot = nc.dram_tensor("dot", [128, 2, n], f8, kind="ExternalOutput").ap()
        dtt = nc.dram_tensor("dtt", [128, 2, 512], f8, kind="ExternalOutput").ap()

    dbgt = {}
    if dbg:
        dbgt = {"dq8": dq8, "dk8": dk8, "dvb": dvb, "dsp": dsp, "dva": dva,
                "dot": dot, "dtt": dtt}
    with tile.TileContext(nc) as tc, ExitStack() as ctx:
        wpool = ctx.enter_context(tc.tile_pool(name="wpool", bufs=1))
        big = ctx.enter_context(tc.tile_pool(name="big", bufs=1))
        xtpool = ctx.enter_context(tc.tile_pool(name="xtpool", bufs=3))
        ttpool = ctx.enter_context(tc.tile_pool(name="ttpool", bufs=12))
        oupool = ctx.enter_context(tc.tile_pool(name="oupool", bufs=4))
        yspool = ctx.enter_context(tc.tile_pool(name="yspool", bufs=3))
        psp = ctx.enter_context(tc.tile_pool(name="psp", bufs=1, space="PSUM"))

        # ---- weights / constants (DMA once) ----
        w_sb = wpool.tile([128, 8, 768], f8)
        w8r = w8.rearrange("(kc p) c -> p kc c", p=128)
        nc.sync.dma_start(out=w_sb[:, :, 0:512], in_=w8r[:, :, 0:512])
        wo_sb = wpool.tile([128, 2, D], f8)
        cbf_sb = wpool.tile([128, n + 512], bf)
        ubc_sb = cbf_sb[:, 0:n]
        mtri_sb = cbf_sb[:, n:n + 128]
        identb_sb = cbf_sb[:, n + 128:n + 256]
        aux_sb = cbf_sb[0:1, n + 256:n + 512]
        def late_const_dmas():
            nc.sync.dma_start(out=cbf_sb, in_=cbf)
            nc.sync.dma_start(out=w_sb[:, :, 512:768], in_=w8r[:, :, 512:768])
            nc.sync.dma_start(out=wo_sb, in_=wo8.rearrange("(c p) d -> p c d", p=128))

        # ---- persistent activations ----
        # q8/k8: [part, m-chunk(head pair), DR-subtile slot, col]; slot 1 is
        # zeroed once so the scores DoubleRow contracts [real | zeros].
        q8 = big.tile([128, 2, 2, n], f8)
        k8 = big.tile([128, 2, 2, n], f8)
        vb16 = big.tile([128, 2, n], bf)       # v (unscaled), transposed layout
        spref = big.tile([128, 2, n], bf)      # prefix sums of v
        vaug = big.tile([128, 2, 8, 2, 128], f8)  # v natural, per (pair-of-heads c, jb)
        outT = big.tile([128, 2, n], f8)       # 16 * attention output, transposed
        nc.gpsimd.memset(q8[:, :, 1, :], 0.0)
        nc.gpsimd.memset(k8[:, :, 1, :], 0.0)

        rep_ctx = tc.For_i(0, reps, 1) if reps > 1 else contextlib.nullcontext()
        ctx.enter_context(rep_ctx)

        def p1_items(c4, ptag="one", pbufs=1):
            """QKV^T chunk c4 as a list of closures (PE filler work)."""
            cols = slice(c4 * 512, (c4 + 1) * 512)
            xtc = xtpool.tile([128, 8, 512], f8, tag="xt", name=f"xtc_{c4}")

            def dma_item():
                eng = nc.scalar if c4 == 0 else nc.sync
                eng.dma_start(
                    out=xtc,
                    in_=xt8.rearrange("(kc p) c -> p kc c", p=128)[:, :, cols])

            def qkv_item(m):
                def run():
                    qps = psp.tile([128, 512], f32, tag=ptag, bufs=pbufs,
                                   name=f"qkv_{c4}_{m}")
                    for kk in range(4):
                        nc.tensor.matmul(out=qps,
                                         lhsT=w_sb[:, 2 * kk:2 * kk + 2, m * 128:(m + 1) * 128],
                                         rhs=xtc[:, 2 * kk:2 * kk + 2, :],
                                         start=(kk == 0), stop=(kk == 3), perf_mode=DR)
                    if m < 2:
                        nc.vector.tensor_copy(out=q8[:, m, 0, cols], in_=qps)
                    elif m < 4:
                        nc.vector.tensor_copy(out=k8[:, m - 2, 0, cols], in_=qps)
                    else:
                        nc.vector.tensor_scalar_mul(out=vb16[:, m - 4, cols], in0=qps,
                                                    scalar1=2.0 ** -5)
                return run

            def sct_item(h):
                def run():
                    ro, c = 64 * (h % 2), h // 2
                    init = 0.0 if c4 == 0 else spref[ro:ro + 64, c, c4 * 512 - 1:c4 * 512]
                    nc.vector.tensor_tensor_scan(out=spref[ro:ro + 64, c, cols],
                                                 data0=vb16[ro:ro + 64, c, cols],
                                                 data1=vb16[ro:ro + 64, c, cols],
                                                 initial=init, op0=ALU.add,
                                                 op1=ALU.bypass)
                    tp = psp.tile([128, 2, 2, 64], bf, tag=ptag, bufs=pbufs,
                                  name=f"tp_{c4}_{h}")
                    for bi in range(4):
                        jb = 4 * c4 + bi
                        nc.tensor.transpose(out=tp[:, bi // 2, bi % 2, :],
                                            in_=vb16[ro:ro + 64, c, jb * 128:(jb + 1) * 128],
                                            identity=identb_sb[ro:ro + 64, ro:ro + 64])
                    nc.vector.tensor_copy(
                        out=vaug[:, c, 2 * c4:2 * c4 + 2, :, ro:ro + 64], in_=tp)
                return run

            # q/k of head-pair 0 first: stage (h0, p0) depends only on m0+m2
            return [dma_item] + [qkv_item(m) for m in (0, 2, 1, 3, 4, 5)] + \
                [sct_item(h) for h in range(4)]

        def p3_items(a, ptag="one", pbufs=1, tail=False):
            """Output projection for token chunk a: 8 half-block closures."""
            items = []
            span = 2
            for g in range(4 // span):
                ib0 = 4 * a + span * g
                ysb = yspool.tile([128, span, 1024], f8, tag="ys",
                                  name=f"ysb_{ib0}")

                def half_item(ib, n2, ysb, ib0=ib0):
                    def run():
                        ypt = psp.tile([128, 512], f32, tag=ptag, bufs=pbufs,
                                       name=f"ypt_{ib}_{n2}")
                        nc.tensor.matmul(out=ypt,
                                         lhsT=outT[:, :, ib * 128:(ib + 1) * 128],
                                         rhs=wo_sb[:, :, n2 * 512:(n2 + 1) * 512],
                                         start=True, stop=True, perf_mode=DR)
                        half = ysb[:, ib - ib0, n2 * 512:(n2 + 1) * 512]
                        if tail and (ib + n2) % 2 == 0:
                            nc.scalar.copy(out=half, in_=ypt)
                        else:
                            nc.vector.tensor_copy(out=half, in_=ypt)
                        if ib == ib0 + span - 1 and n2 == 1:
                            nc.sync.dma_start(
                                out=yout[ib0 * 128:(ib0 + span) * 128, :].rearrange(
                                    "(i p) d -> p i d", p=128),
                                in_=ysb)
                    return run

                for ib in range(ib0, ib0 + span):
                    items += [half_item(ib, 0, ysb), half_item(ib, 1, ysb)]
            return items

        def stage_units(ic, last=False):
            """Attention units for output chunk ic. Each unit = (front, back):
            front = scores+masks+silu, back = attn*v matmul (+finalize on the
            head's last pair). The weaver emits back one unit late so the PE
            stream never blocks on the silu it just requested."""
            base = ic * 512
            pmax = min(7, 2 * ic + 1)
            units = []
            ap_ref = {}
            for h in range(4):
                ro, c = 64 * (h % 2), h // 2
                for p in range(pmax + 1):
                    diag = (p // 2 == ic)
                    lo = 256 * (p % 2) if diag else 0
                    tt = ttpool.tile([128, 2, 512], f8, tag="tt",
                                     name=f"tt_{h}_{p}_{ic}")

                    def front(h=h, p=p, ro=ro, c=c, lo=lo, diag=diag, tt=tt):
                        sps = psp.tile([128, 2, 512], f32, tag="big2", bufs=3,
                                       name=f"sps_{h}_{p}_{ic}")
                        nc.tensor.matmul(out=sps[:, 0, lo:512],
                                         lhsT=k8[ro:ro + 64, c, :, 2 * p * 128:(2 * p + 1) * 128],
                                         rhs=q8[ro:ro + 64, c, :, base + lo:base + 512],
                                         start=True, stop=not diag, perf_mode=DR,
                                         skip_group_check=True)
                        if diag:
                            nc.tensor.matmul(out=sps[:, 0, lo:lo + 128], lhsT=mtri_sb,
                                             rhs=identb_sb, start=False, stop=True,
                                             skip_group_check=True)
                            nc.tensor.matmul(out=sps[:, 1, lo:lo + 128],
                                             lhsT=aux_sb[0:1, 0:128],
                                             rhs=aux_sb[0:1, 128:256],
                                             start=True, stop=True,
                                             skip_group_check=True)
                            nc.tensor.matmul(out=sps[:, 1, lo + 128:512],
                                             lhsT=k8[ro:ro + 64, c, :, (2 * p + 1) * 128:(2 * p + 2) * 128],
                                             rhs=q8[ro:ro + 64, c, :, base + lo + 128:base + 512],
                                             start=True, stop=False, perf_mode=DR,
                                             skip_group_check=True)
                            nc.tensor.matmul(out=sps[:, 1, lo + 128:lo + 256],
                                             lhsT=mtri_sb, rhs=identb_sb,
                                             start=False, stop=True,
                                             skip_group_check=True)
                        else:
                            nc.tensor.matmul(out=sps[:, 1, 0:512],
                                             lhsT=k8[ro:ro + 64, c, :, (2 * p + 1) * 128:(2 * p + 2) * 128],
                                             rhs=q8[ro:ro + 64, c, :, base:base + 512],
                                             start=True, stop=True, perf_mode=DR,
                                             skip_group_check=True)
                        nc.scalar.activation(out=tt[:, :, lo:512],
                                             in_=sps[:, :, lo:512],
                                             func=AF.Silu, scale=2.0 ** -13)
                        if dbg and h == 0 and p == 0 and ic == 0:
                            nc.sync.dma_start(out=dbgt["dtt"], in_=tt)

                    def back(h=h, p=p, ro=ro, c=c, lo=lo, tt=tt):
                        if p == 0:
                            ap_ref[h] = psp.tile([128, 512], f32, tag="ap", bufs=1,
                                                 name=f"ap_{h}_{ic}")
                        nc.tensor.matmul(out=ap_ref[h][:, lo:512],
                                         lhsT=vaug[:, c, p, :, :],
                                         rhs=tt[:, :, lo:512],
                                         start=(p == 0), stop=(p == pmax),
                                         perf_mode=DR, skip_group_check=True)
                        if p == pmax:
                            # out = (prefix(v) + ap/n) * 16/(i+1), fp8 (x16).
                            # The very last finalize is split in halves on DVE
                            # so the trailing out-proj starts half a tile early.
                            crit = last and h == 3
                            ou = oupool.tile([128, 512], bf, tag="ou",
                                             name=f"ou_{h}_{ic}")
                            mul_eng = nc.vector if crit else nc.gpsimd
                            for s0, s1 in ([(0, 256), (256, 512)] if crit
                                           else [(0, 512)]):
                                nc.vector.scalar_tensor_tensor(
                                    out=ou[ro:ro + 64, s0:s1],
                                    in0=ap_ref[h][ro:ro + 64, s0:s1],
                                    scalar=1.0 / n,
                                    in1=spref[ro:ro + 64, c, base + s0:base + s1],
                                    op0=ALU.mult, op1=ALU.add)
                                mul_eng.tensor_mul(
                                    out=outT[ro:ro + 64, c, base + s0:base + s1],
                                    in0=ou[ro:ro + 64, s0:s1],
                                    in1=ubc_sb[ro:ro + 64, base + s0:base + s1])

                    units.append((front, back))
            return units

        pending_back = [None]

        def weave(units, fillers):
            """Emit units with backs delayed one unit; spread fillers evenly."""
            nf, nu = len(fillers), max(1, len(units))
            fi = 0
            for i, (front, back) in enumerate(units):
                front()
                if pending_back[0] is not None:
                    pending_back[0]()
                pending_back[0] = back
                want = min(nf, (i + 1) * nf // max(1, nu - 2))
                while fi < want:
                    fillers[fi]()
                    fi += 1
            while fi < nf:
                fillers[fi]()
                fi += 1

        # Stage 0 is folded into P1(0): fronts only need the m0/m2 (and
        # m1/m3) drains, so they start as soon as those chains land; backs
        # (which need vaug/scan) are deferred past the sct items, interleaved
        # with P1(1). This pulls the first silu ~5us earlier.
        p10 = p1_items(0, ptag="big2", pbufs=3)
        for item in p10[:3]:   # xtc DMA, qkv m0, qkv m2
            item()
        late_const_dmas()
        su0 = stage_units(0)
        rest = p10[3:]
        for i, (front, _) in enumerate(su0):
            front()
            if i < len(rest):
                rest[i]()
        for item in rest[len(su0):]:
            item()
        p11 = p1_items(1)
        for i, (_, back) in enumerate(su0):
            back()
            j0, j1 = i * len(p11) // len(su0), (i + 1) * len(p11) // len(su0)
            for item in p11[j0:j1]:
                item()
        for ic in range(1, nstg):
            fillers = []
            if ic + 1 < nstg:
                fillers += p1_items(ic + 1)
            fillers += p3_items(ic - 1)
            weave(stage_units(ic, last=(ic == nstg - 1)), fillers)
        if pending_back[0] is not None:
            pending_back[0]()
        for item in p3_items(nstg - 1, ptag="big2", pbufs=3, tail=True):
            item()
        if dbg:
            nc.sync.dma_start(out=dbgt["dq8"], in_=q8)
            nc.sync.dma_start(out=dbgt["dk8"], in_=k8)
            nc.sync.dma_start(out=dbgt["dvb"], in_=vb16)
            nc.sync.dma_start(out=dbgt["dsp"], in_=spref)
            nc.sync.dma_start(out=dbgt["dva"], in_=vaug)
            nc.sync.dma_start(out=dbgt["dot"], in_=outT)

    nc.compile()
    return nc


def prep_in_maps(x, ln_g, ln_b, w_qkv, w_out, n=N_FULL, n_batches=B):
    """Host-side prep: LayerNorm, weight fold/reorder, fp8 casts, per-core dicts."""
    import ml_dtypes
    f8 = ml_dtypes.float8_e4m3fn
    bf16 = ml_dtypes.bfloat16

    x = np.asarray(x, np.float32)
    mu = x.mean(-1, keepdims=True)
    var = ((x - mu) ** 2).mean(-1, keepdims=True)
    xn = (x - mu) / np.sqrt(var + EPS) * np.asarray(ln_g, np.float32) \
        + np.asarray(ln_b, np.float32)
    w_qkv = np.asarray(w_qkv, np.float32)
    w_out = np.asarray(w_out, np.float32)

    idx = np.arange(128)
    # packed bf16 constants [128, n+512]: ubc | mtri | identb | aux(-B, ones)
    cbf = np.zeros((128, n + 512), np.float32)
    cbf[:, 0:n] = 16.0 / np.arange(1, n + 1, dtype=np.float64)[None, :]
    cbf[:, n:n + 128] = np.where(idx[None, :] > idx[:, None], -NEGB, 0.0)
    cbf[:, n + 128:n + 256] = np.eye(128)
    cbf[0, n + 256:n + 384] = -NEGB
    cbf[0, n + 384:n + 512] = 1.0
    cbf = cbf.astype(bf16)

    in_maps = []
    for d in range(2 * n_batches):
        b, g = divmod(d, 2)
        # m-chunk neuron order: m0 q h01 | m1 q h23 | m2 k h01 | m3 k h23 | m4 v h01 | m5 v h23
        order = []
        for off in (0, 64, 128):  # q, k, v row offsets within a head's 256 rows
            for c in range(2):
                for i in (0, 1):
                    hh = g * 4 + 2 * c + i
                    order += list(range(hh * 256 + off, hh * 256 + off + 64))
        w8 = np.ascontiguousarray((w_qkv[order, :] * 32.0).T).astype(f8)  # [1024, 768]
        wo8 = np.ascontiguousarray(w_out[:, g * 256:(g + 1) * 256].T * 16.0).astype(f8)
        in_maps.append({
            "xt8": np.ascontiguousarray(xn[b].T).astype(f8),
            "w8": w8,
            "wo8": wo8,
            "cbf": cbf,
        })
    return in_maps


_cached_nc = None


def kernel(x, attention_mask, ln_g, ln_b, w_qkv, b_qkv, w_out, b_out):
    """Full-input entry point: shards across 8 NeuronCores, returns full output."""
    global _cached_nc
    from concourse.bass_utils import run_bass_kernel_spmd

    if _cached_nc is None:
        _cached_nc = build_nc(N_FULL)
    nc = _cached_nc

    in_maps = prep_in_maps(x, ln_g, ln_b, w_qkv, w_out)
    res = run_bass_kernel_spmd(nc, in_maps, core_ids=list(range(NCORES)))

    y = np.asarray(x, np.float32) + np.asarray(b_out, np.float32)[None, None, :]
    for d in range(NCORES):
        y[d // 2] += res.results[d]["yout"].astype(np.float32) * 2.0 ** -8
    return y
